# revision 30
# baseline (speedup 1.0000x reference)
"""Trainium2 Bass kernel for nn_Cache_28071906246843 (retrieval_knn).

reference semantics:
    q = h_t[cache_words]                         # [C, D] gather
    dist = sqrt(sum((cache_h - q)**2, -1))       # [C]
    vals = exp(dist / 32.0)                      # [C]
    cache_p = segment_sum(vals, cache_words, V)  # [V]
    out = log_softmax(cache_p[None, :])          # [1, V]

v5 (main path): cache elements are sorted by word id and split into 8
contiguous shards of 16384 elements, one per NeuronCore.  Using
  d2[e] = ||ch_e||^2 + ||h_t[w_e]||^2 - 2 * ch_e . h_t[w_e]
(norms precomputed on the host), the only O(C*D) device work is the
cross-term dot product.  Per supertile of 256 sorted elements the <=128
distinct h_t rows W and the element block CH are shipped pre-transposed
in fp8; the TensorEngine computes M = W @ CH^T ([128w, 256e]) as 4
DoubleRow-fp8 matmuls (contraction 2x128 each), Scalar/Vector copy the
PSUM tile to fp16 and it is DMA'd back.  The host then selects
dot[e] = M[rel[e], e], applies sqrt/exp, segment-sums into [V] and
takes the log_softmax (same O(C)+O(V) host tail as before).
DMA queues: cht on Sync+GpSimd, W on Scalar+Vector, M out on
Sync+GpSimd; all transfers are >=512B per partition line.

v4/v1 device paths are kept as fallbacks (not used for this input).
"""

import sys

import numpy as np

if "/opt/trn_rl_repo" not in sys.path:
    sys.path.insert(0, "/opt/trn_rl_repo")

import ml_dtypes

import concourse.bass as bass
import concourse.tile as tile
from concourse import bacc, mybir
from concourse.bass_utils import run_bass_kernel_spmd

V, D, C = 50257, 1024, 131072
NCORES = 8
CSH = C // NCORES  # 16384 elements per core
P = 128            # SBUF partitions
NT = CSH // P      # 128 tiles per core
SMOOTH = 32.0

SUP = 2            # element-tiles per supertile
NSUP = NT // SUP   # 64 supertiles per core
SUPW = SUP * P     # 256 elements per supertile

FP8 = ml_dtypes.float8_e4m3


# ---------------------------------------------------------------- v5 ----


def build_nc_v5() -> bass.Bass:
    """Per-core SPMD program: per supertile, M = W @ CH^T via DoubleRow fp8
    matmuls; PSUM -> fp16 SBUF (Scalar/Vector alternating) -> DRAM."""
    nc = bacc.Bacc(
        "TRN2", target_bir_lowering=False, debug=False, num_devices=NCORES
    )
    # cht batched 4 sups per DMA, wt batched 8 sups (4 pairs) per DMA,
    # M out fp8 batched 8 sups per DMA: amortizes the per-DMA overhead.
    # No DMAs on the Sync (SP) queue: it serves as the semaphore-relay
    # hub, and DMAs enqueued there delay every cross-engine notification.
    cht = nc.dram_tensor(
        "cht", [NSUP // 4, P, 4, 4, 2, SUPW], mybir.dt.float8e4,
        kind="ExternalInput",
    )
    wtp = nc.dram_tensor(
        "wtp", [NSUP // 8, P, 4, 2, 4, 2, P], mybir.dt.float8e4,
        kind="ExternalInput",
    )
    m8 = nc.dram_tensor(
        "m8", [NSUP // 8, P, 8, SUPW], mybir.dt.float8e4, kind="ExternalOutput"
    )
    cht_ap = cht.ap()
    wtp_ap = wtp.ap()
    m8_ap = m8.ap()
    DR = mybir.MatmulPerfMode.DoubleRow

    PFB = 3  # cht prefetch distance in 4-sup batches

    with tile.TileContext(nc) as tc:
        with (
            tc.tile_pool(name="chp", bufs=5) as chp,
            tc.tile_pool(name="wpool", bufs=3) as wpool,
            tc.tile_pool(name="stage", bufs=8) as stage,
            tc.tile_pool(name="psum", bufs=8, space="PSUM") as psump,
        ):
            pending = []

            def flush_out():
                o8, ost = pending.pop(0)
                nc.scalar.dma_start(out=m8_ap[o8], in_=ost[:])

            ch_tiles = {}
            wt_tiles = {}

            def fetch(b4, split=False):
                """Issue the cht DMA for 4-sup batch b4 (alternating queue)."""
                if b4 >= NSUP // 4:
                    return
                ch_sb = chp.tile(
                    [P, 4, 4, 2, SUPW], mybir.dt.float8e4, tag="ch"
                )
                eng = nc.scalar if b4 % 2 == 0 else nc.gpsimd
                if split:
                    # per-sup pieces so the first matmul starts sooner
                    for v in range(4):
                        eng.dma_start(
                            out=ch_sb[:, v], in_=cht_ap[b4, :, v]
                        )
                else:
                    eng.dma_start(out=ch_sb[:], in_=cht_ap[b4])
                ch_tiles[b4] = ch_sb

            def fetch_wt(o, split=False):
                """Issue the wt DMA for 8-sup octet o (alternating queue)."""
                if o >= NSUP // 8:
                    return
                wt_sb = wpool.tile(
                    [P, 4, 2, 4, 2, P], mybir.dt.float8e4, tag="wt"
                )
                eng = nc.gpsimd if o % 2 == 0 else nc.scalar
                if split:
                    for pr in range(4):
                        eng.dma_start(
                            out=wt_sb[:, pr], in_=wtp_ap[o, :, pr]
                        )
                else:
                    eng.dma_start(out=wt_sb[:], in_=wtp_ap[o])
                wt_tiles[o] = wt_sb

            # the first wt + first cht go first, split into fine pieces on
            # different queues, so compute starts after ~0.5MB of
            # transfers; DMA bandwidth is the global bottleneck.
            fetch_wt(0, split=True)
            fetch(0, split=True)
            fetch(1)
            fetch_wt(1)
            for b4 in range(2, PFB):
                fetch(b4)

            for o in range(NSUP // 8):
                st = stage.tile([P, 8, SUPW], mybir.dt.float8e4, tag="st")
                for half in range(2):  # 4-sup cht batch within the octet
                    b4 = 2 * o + half
                    fetch(b4 + PFB)
                    if half == 1:
                        fetch_wt(o + 2)
                    ch_sb = ch_tiles.pop(b4)
                    wt_sb = wt_tiles[o] if half == 0 else wt_tiles.pop(o)

                    for g in range(2):  # 2-sup psum group
                        mp = psump.tile(
                            [P, 2 * SUPW], mybir.dt.float32, tag="mp"
                        )
                        for u2 in range(2):  # sup within the group
                            v = 2 * g + u2  # sup within the 4-batch
                            k = 4 * half + v  # sup within the octet
                            pair, u = divmod(k, 2)
                            for j in range(4):
                                nc.tensor.matmul(
                                    out=mp[:, u2 * SUPW : (u2 + 1) * SUPW],
                                    lhsT=wt_sb[:, pair, u, j, :, :],
                                    rhs=ch_sb[:, v, j, :, :],
                                    start=(j == 0),
                                    stop=(j == 3),
                                    perf_mode=DR,
                                )
                        kk = 4 * half + 2 * g
                        nc.vector.tensor_scalar_add(
                            st[:, kk : kk + 2, :], mp[:], 0.0
                        )
                    if half == 0 and len(pending) >= 1:
                        flush_out()
                pending.append((o, st))
            while pending:
                flush_out()
    nc.compile()
    return nc


def prep_v5(cw_sorted):
    """Per-core supertile metadata: padded distinct-word ids and, per
    element, the index of its word within its supertile's distinct list.
    Returns None if any supertile has >128 distinct words."""
    widx_all, rel_all = [], []
    for c in range(NCORES):
        shard = cw_sorted[c * CSH : (c + 1) * CSH]
        widx = np.empty((NSUP, P), np.int32)
        rel = np.empty(CSH, np.int32)
        for s in range(NSUP):
            seg = shard[s * SUPW : (s + 1) * SUPW]
            uw = np.unique(seg)
            if len(uw) > P:
                return None
            widx[s, : len(uw)] = uw
            widx[s, len(uw) :] = uw[-1]
            rel[s * SUPW : (s + 1) * SUPW] = np.searchsorted(uw, seg)
        widx_all.append(widx)
        rel_all.append(rel)
    return widx_all, rel_all


def make_in_maps_v5(h_t, ch_sorted, widx_all):
    ht8 = h_t.astype(FP8)
    in_maps = []
    for c in range(NCORES):
        ch8 = ch_sorted[c * CSH : (c + 1) * CSH].astype(FP8)
        # cht[b4, p, v, j, i, e] = ch8[(4*b4+v)*256+e, (2j+i)*128+p]
        cht = np.ascontiguousarray(
            ch8.reshape(NSUP, SUPW, 4, 2, P)
            .transpose(0, 4, 2, 3, 1)
            .reshape(NSUP // 4, 4, P, 4, 2, SUPW)
            .transpose(0, 2, 1, 3, 4, 5)
        )
        w8 = ht8[widx_all[c]]  # [NSUP, 128w, 1024]
        # wt[s, p, j, i, w] = ht8[widx[s, w], (2j+i)*128+p], s = 8o+2h+u
        wt = w8.reshape(NSUP, P, 4, 2, P).transpose(0, 4, 2, 3, 1)
        wtp = np.ascontiguousarray(
            wt.reshape(NSUP // 8, 4, 2, P, 4, 2, P)
            .transpose(0, 3, 1, 2, 4, 5, 6)
        )
        in_maps.append({"cht": cht, "wtp": wtp})
    return in_maps


def finish_v5(m8_list, rel_all, h_t, ch_sorted, cw_sorted):
    """Select dot[e] = M[rel[e], e], rebuild d2 from host norms, exp."""
    nht2 = np.einsum("ij,ij->i", h_t, h_t, dtype=np.float32)
    e_idx = np.arange(CSH)
    s_idx = e_idx // SUPW
    vals = []
    for c in range(NCORES):
        M = np.asarray(m8_list[c], dtype=np.float32)  # [8, 128, 8, 256]
        dot = M[s_idx // 8, rel_all[c], s_idx % 8, e_idx % SUPW]
        ch = ch_sorted[c * CSH : (c + 1) * CSH]
        nch2 = np.einsum("ij,ij->i", ch, ch, dtype=np.float32)
        d2 = nch2 + nht2[cw_sorted[c * CSH : (c + 1) * CSH]] - 2.0 * dot
        vals.append(np.exp(np.sqrt(np.maximum(d2, 0.0)) / SMOOTH))
    return np.concatenate(vals)


# ------------------------------------------------- v1 fallback ----------


def build_nc(nt: int = NT, v: int = V, d: int = D) -> bass.Bass:
    """Per-element-gather fallback (slow, always correct)."""
    nc = bacc.Bacc(
        "TRN2", target_bir_lowering=False, debug=False, num_devices=NCORES
    )
    ht = nc.dram_tensor("ht", [v, d], mybir.dt.float32, kind="ExternalInput")
    ch = nc.dram_tensor("ch", [nt * P, d], mybir.dt.float32, kind="ExternalInput")
    cw = nc.dram_tensor("cw", [P, nt], mybir.dt.int32, kind="ExternalInput")
    vals = nc.dram_tensor("vals", [P, nt], mybir.dt.float32, kind="ExternalOutput")

    ch_t = ch.ap().rearrange("(t p) d -> t p d", p=P)

    with tile.TileContext(nc) as tc:
        with (
            tc.tile_pool(name="io", bufs=6) as io,
            tc.tile_pool(name="scratch", bufs=2) as scratch,
            tc.tile_pool(name="persist", bufs=1) as persist,
        ):
            cw_sb = persist.tile([P, nt], mybir.dt.int32)
            nc.sync.dma_start(out=cw_sb[:], in_=cw.ap())
            vals_sb = persist.tile([P, nt], mybir.dt.float32)
            d2_all = persist.tile([P, nt], mybir.dt.float32)

            for t in range(nt):
                ch_tile = io.tile([P, d], mybir.dt.float32, tag="ch")
                nc.sync.dma_start(out=ch_tile[:], in_=ch_t[t])

                q_tile = io.tile([P, d], mybir.dt.float32, tag="q")
                nc.gpsimd.indirect_dma_start(
                    out=q_tile[:],
                    out_offset=None,
                    in_=ht.ap(),
                    in_offset=bass.IndirectOffsetOnAxis(
                        ap=cw_sb[:, t : t + 1], axis=0
                    ),
                )

                d_tile = io.tile([P, d], mybir.dt.float32, tag="d")
                nc.vector.tensor_tensor(
                    out=d_tile[:],
                    in0=ch_tile[:],
                    in1=q_tile[:],
                    op=mybir.AluOpType.subtract,
                )

                sq_tile = scratch.tile([P, d], mybir.dt.float32, tag="sq")
                nc.scalar.activation(
                    out=sq_tile[:],
                    in_=d_tile[:],
                    func=mybir.ActivationFunctionType.Square,
                    accum_out=d2_all[:, t : t + 1],
                )

            dist_all = persist.tile([P, nt], mybir.dt.float32)
            nc.scalar.activation(
                out=dist_all[:],
                in_=d2_all[:],
                func=mybir.ActivationFunctionType.Sqrt,
            )
            nc.scalar.activation(
                out=vals_sb[:],
                in_=dist_all[:],
                func=mybir.ActivationFunctionType.Exp,
                scale=1.0 / SMOOTH,
            )

            nc.sync.dma_start(out=vals.ap(), in_=vals_sb[:])
    nc.compile()
    return nc


def make_in_maps(h_t, ch_sorted, cw_sorted):
    in_maps = []
    for c in range(NCORES):
        sl = slice(c * CSH, (c + 1) * CSH)
        in_maps.append(
            {
                "ht": h_t,
                "ch": ch_sorted[sl],
                "cw": np.ascontiguousarray(cw_sorted[sl].reshape(NT, P).T),
            }
        )
    return in_maps


# ---------------------------------------------------------- shared ------


def finish_on_host(vals_sorted, cw_sorted):
    """segment-sum + log_softmax (tiny O(C)+O(V) work)."""
    p = np.bincount(cw_sorted, weights=vals_sorted.astype(np.float64), minlength=V)
    m = p.max()
    lse = m + np.log(np.exp(p - m).sum())
    return (p - lse).astype(np.float32)[None, :]


def _prep(h_t, cache_h, cache_words):
    h_t = np.ascontiguousarray(np.asarray(h_t), dtype=np.float32)
    cache_h = np.ascontiguousarray(np.asarray(cache_h), dtype=np.float32)
    cw = np.asarray(cache_words).astype(np.int32)
    order = np.argsort(cw, kind="stable")
    return h_t, cache_h[order], cw[order]


def run_device(h_t, ch_sorted, cw_sorted, force_v1=False, verbose=False):
    """Compile + run the SPMD program; returns per-element vals (sorted order)."""
    import time as _time

    _t0 = _time.time()
    v5 = None if force_v1 else prep_v5(cw_sorted)
    if v5 is not None:
        widx_all, rel_all = v5
        nc = build_nc_v5()
        in_maps = make_in_maps_v5(h_t, ch_sorted, widx_all)
    else:
        nc = build_nc()
        in_maps = make_in_maps(h_t, ch_sorted, cw_sorted)
    if verbose:
        print(f"[run_device] build+prep: {_time.time() - _t0:.1f}s")
    _t1 = _time.time()
    res = run_bass_kernel_spmd(nc, in_maps, core_ids=list(range(NCORES)))
    if verbose:
        print(f"[run_device] compile+exec: {_time.time() - _t1:.1f}s")
    if v5 is not None:
        return finish_v5(
            [r["m8"] for r in res.results], rel_all, h_t, ch_sorted, cw_sorted
        )
    return np.concatenate([r["vals"].T.reshape(-1) for r in res.results])


def kernel(h_t, cache_h, cache_words):
    h_t, ch_sorted, cw_sorted = _prep(h_t, cache_h, cache_words)
    vals_sorted = run_device(h_t, ch_sorted, cw_sorted)
    return finish_on_host(vals_sorted, cw_sorted)


# revision 32
# speedup vs baseline: 1.1051x; 1.1051x over previous
"""Trainium2 Bass kernel for nn_Cache_28071906246843 (retrieval_knn).

reference semantics:
    q = h_t[cache_words]                         # [C, D] gather
    dist = sqrt(sum((cache_h - q)**2, -1))       # [C]
    vals = exp(dist / 32.0)                      # [C]
    cache_p = segment_sum(vals, cache_words, V)  # [V]
    out = log_softmax(cache_p[None, :])          # [1, V]

v5 (main path): cache elements are sorted by word id and split into 8
contiguous shards of 16384 elements, one per NeuronCore.  Using
  d2[e] = ||ch_e||^2 + ||h_t[w_e]||^2 - 2 * ch_e . h_t[w_e]
(norms precomputed on the host), the only O(C*D) device work is the
cross-term dot product.  Per supertile of 256 sorted elements the <=128
distinct h_t rows W and the element block CH are shipped pre-transposed
in fp8; the TensorEngine computes M = W @ CH^T ([128w, 256e]) as 4
DoubleRow-fp8 matmuls (contraction 2x128 each), Scalar/Vector copy the
PSUM tile to fp16 and it is DMA'd back.  The host then selects
dot[e] = M[rel[e], e], applies sqrt/exp, segment-sums into [V] and
takes the log_softmax (same O(C)+O(V) host tail as before).
DMA queues: cht on Sync+GpSimd, W on Scalar+Vector, M out on
Sync+GpSimd; all transfers are >=512B per partition line.

v4/v1 device paths are kept as fallbacks (not used for this input).
"""

import sys

import numpy as np

if "/opt/trn_rl_repo" not in sys.path:
    sys.path.insert(0, "/opt/trn_rl_repo")

import ml_dtypes

import concourse.bass as bass
import concourse.tile as tile
from concourse import bacc, mybir
from concourse.bass_utils import run_bass_kernel_spmd

V, D, C = 50257, 1024, 131072
NCORES = 8
CSH = C // NCORES  # 16384 elements per core
P = 128            # SBUF partitions
NT = CSH // P      # 128 tiles per core
SMOOTH = 32.0

SUP = 2            # element-tiles per supertile
NSUP = NT // SUP   # 64 supertiles per core
SUPW = SUP * P     # 256 elements per supertile

FP8 = ml_dtypes.float8_e4m3


# ---------------------------------------------------------------- v5 ----


def build_nc_v5() -> bass.Bass:
    """Per-core SPMD program: per supertile, M = W @ CH^T via DoubleRow fp8
    matmuls; PSUM -> fp16 SBUF (Scalar/Vector alternating) -> DRAM."""
    nc = bacc.Bacc(
        "TRN2", target_bir_lowering=False, debug=False, num_devices=NCORES
    )
    # cht batched 4 sups per DMA, wt batched 8 sups (4 pairs) per DMA,
    # M out fp8 batched 8 sups per DMA: amortizes the per-DMA overhead.
    # No DMAs on the Sync (SP) queue: it serves as the semaphore-relay
    # hub, and DMAs enqueued there delay every cross-engine notification.
    cht = nc.dram_tensor(
        "cht", [NSUP // 4, P, 4, 4, 2, SUPW], mybir.dt.float8e4,
        kind="ExternalInput",
    )
    wtp = nc.dram_tensor(
        "wtp", [NSUP // 8, P, 4, 2, 4, 2, P], mybir.dt.float8e4,
        kind="ExternalInput",
    )
    m8 = nc.dram_tensor(
        "m8", [NSUP // 8, P, 8, SUPW], mybir.dt.float8e4, kind="ExternalOutput"
    )
    cht_ap = cht.ap()
    wtp_ap = wtp.ap()
    m8_ap = m8.ap()
    DR = mybir.MatmulPerfMode.DoubleRow

    PFB = 3  # cht prefetch distance in 4-sup batches

    with tile.TileContext(nc) as tc:
        with (
            tc.tile_pool(name="chp", bufs=5) as chp,
            tc.tile_pool(name="wpool", bufs=3) as wpool,
            tc.tile_pool(name="stage", bufs=8) as stage,
            tc.tile_pool(name="psum", bufs=8, space="PSUM") as psump,
        ):
            pending = []

            def flush_out():
                o8, ost = pending.pop(0)
                nc.scalar.dma_start(out=m8_ap[o8], in_=ost[:])

            ch_tiles = {}
            wt_tiles = {}

            def fetch(b4, split=False):
                """Issue the cht DMA for 4-sup batch b4 (alternating queue)."""
                if b4 >= NSUP // 4:
                    return
                ch_sb = chp.tile(
                    [P, 4, 4, 2, SUPW], mybir.dt.float8e4, tag="ch"
                )
                eng = nc.gpsimd if b4 % 2 == 0 else nc.scalar
                if split:
                    # per-sup pieces so the first matmul starts sooner
                    for v in range(4):
                        eng.dma_start(
                            out=ch_sb[:, v], in_=cht_ap[b4, :, v]
                        )
                else:
                    eng.dma_start(out=ch_sb[:], in_=cht_ap[b4])
                ch_tiles[b4] = ch_sb

            def fetch_wt(o, split=False):
                """Issue the wt DMA for 8-sup octet o (alternating queue)."""
                if o >= NSUP // 8:
                    return
                wt_sb = wpool.tile(
                    [P, 4, 2, 4, 2, P], mybir.dt.float8e4, tag="wt"
                )
                eng = nc.gpsimd if o % 2 == 0 else nc.scalar
                if split:
                    for pr in range(4):
                        eng.dma_start(
                            out=wt_sb[:, pr], in_=wtp_ap[o, :, pr]
                        )
                else:
                    eng.dma_start(out=wt_sb[:], in_=wtp_ap[o])
                wt_tiles[o] = wt_sb

            fetch_wt(0)
            fetch_wt(1)
            for b4 in range(PFB):
                fetch(b4)

            for o in range(NSUP // 8):
                st = stage.tile([P, 8, SUPW], mybir.dt.float8e4, tag="st")
                for half in range(2):  # 4-sup cht batch within the octet
                    b4 = 2 * o + half
                    fetch(b4 + PFB)
                    if half == 1:
                        fetch_wt(o + 2)
                    ch_sb = ch_tiles.pop(b4)
                    wt_sb = wt_tiles[o] if half == 0 else wt_tiles.pop(o)

                    for g in range(2):  # 2-sup psum group
                        mp = psump.tile(
                            [P, 2 * SUPW], mybir.dt.float32, tag="mp"
                        )
                        for u2 in range(2):  # sup within the group
                            v = 2 * g + u2  # sup within the 4-batch
                            k = 4 * half + v  # sup within the octet
                            pair, u = divmod(k, 2)
                            for j in range(4):
                                nc.tensor.matmul(
                                    out=mp[:, u2 * SUPW : (u2 + 1) * SUPW],
                                    lhsT=wt_sb[:, pair, u, j, :, :],
                                    rhs=ch_sb[:, v, j, :, :],
                                    start=(j == 0),
                                    stop=(j == 3),
                                    perf_mode=DR,
                                )
                        kk = 4 * half + 2 * g
                        nc.vector.tensor_scalar_add(
                            st[:, kk : kk + 2, :], mp[:], 0.0
                        )
                    if half == 0 and len(pending) >= 1:
                        flush_out()
                pending.append((o, st))
            while pending:
                flush_out()
    nc.compile()
    return nc


def prep_v5(cw_sorted):
    """Per-core supertile metadata: padded distinct-word ids and, per
    element, the index of its word within its supertile's distinct list.
    Returns None if any supertile has >128 distinct words."""
    widx_all, rel_all = [], []
    for c in range(NCORES):
        shard = cw_sorted[c * CSH : (c + 1) * CSH]
        widx = np.empty((NSUP, P), np.int32)
        rel = np.empty(CSH, np.int32)
        for s in range(NSUP):
            seg = shard[s * SUPW : (s + 1) * SUPW]
            uw = np.unique(seg)
            if len(uw) > P:
                return None
            widx[s, : len(uw)] = uw
            widx[s, len(uw) :] = uw[-1]
            rel[s * SUPW : (s + 1) * SUPW] = np.searchsorted(uw, seg)
        widx_all.append(widx)
        rel_all.append(rel)
    return widx_all, rel_all


def make_in_maps_v5(h_t, ch_sorted, widx_all):
    ht8 = h_t.astype(FP8)
    in_maps = []
    for c in range(NCORES):
        ch8 = ch_sorted[c * CSH : (c + 1) * CSH].astype(FP8)
        # cht[b4, p, v, j, i, e] = ch8[(4*b4+v)*256+e, (2j+i)*128+p]
        cht = np.ascontiguousarray(
            ch8.reshape(NSUP, SUPW, 4, 2, P)
            .transpose(0, 4, 2, 3, 1)
            .reshape(NSUP // 4, 4, P, 4, 2, SUPW)
            .transpose(0, 2, 1, 3, 4, 5)
        )
        w8 = ht8[widx_all[c]]  # [NSUP, 128w, 1024]
        # wt[s, p, j, i, w] = ht8[widx[s, w], (2j+i)*128+p], s = 8o+2h+u
        wt = w8.reshape(NSUP, P, 4, 2, P).transpose(0, 4, 2, 3, 1)
        wtp = np.ascontiguousarray(
            wt.reshape(NSUP // 8, 4, 2, P, 4, 2, P)
            .transpose(0, 3, 1, 2, 4, 5, 6)
        )
        in_maps.append({"cht": cht, "wtp": wtp})
    return in_maps


def finish_v5(m8_list, rel_all, h_t, ch_sorted, cw_sorted):
    """Select dot[e] = M[rel[e], e], rebuild d2 from host norms, exp."""
    nht2 = np.einsum("ij,ij->i", h_t, h_t, dtype=np.float32)
    e_idx = np.arange(CSH)
    s_idx = e_idx // SUPW
    vals = []
    for c in range(NCORES):
        M = np.asarray(m8_list[c], dtype=np.float32)  # [8, 128, 8, 256]
        dot = M[s_idx // 8, rel_all[c], s_idx % 8, e_idx % SUPW]
        ch = ch_sorted[c * CSH : (c + 1) * CSH]
        nch2 = np.einsum("ij,ij->i", ch, ch, dtype=np.float32)
        d2 = nch2 + nht2[cw_sorted[c * CSH : (c + 1) * CSH]] - 2.0 * dot
        vals.append(np.exp(np.sqrt(np.maximum(d2, 0.0)) / SMOOTH))
    return np.concatenate(vals)


# ------------------------------------------------- v1 fallback ----------


def build_nc(nt: int = NT, v: int = V, d: int = D) -> bass.Bass:
    """Per-element-gather fallback (slow, always correct)."""
    nc = bacc.Bacc(
        "TRN2", target_bir_lowering=False, debug=False, num_devices=NCORES
    )
    ht = nc.dram_tensor("ht", [v, d], mybir.dt.float32, kind="ExternalInput")
    ch = nc.dram_tensor("ch", [nt * P, d], mybir.dt.float32, kind="ExternalInput")
    cw = nc.dram_tensor("cw", [P, nt], mybir.dt.int32, kind="ExternalInput")
    vals = nc.dram_tensor("vals", [P, nt], mybir.dt.float32, kind="ExternalOutput")

    ch_t = ch.ap().rearrange("(t p) d -> t p d", p=P)

    with tile.TileContext(nc) as tc:
        with (
            tc.tile_pool(name="io", bufs=6) as io,
            tc.tile_pool(name="scratch", bufs=2) as scratch,
            tc.tile_pool(name="persist", bufs=1) as persist,
        ):
            cw_sb = persist.tile([P, nt], mybir.dt.int32)
            nc.sync.dma_start(out=cw_sb[:], in_=cw.ap())
            vals_sb = persist.tile([P, nt], mybir.dt.float32)
            d2_all = persist.tile([P, nt], mybir.dt.float32)

            for t in range(nt):
                ch_tile = io.tile([P, d], mybir.dt.float32, tag="ch")
                nc.sync.dma_start(out=ch_tile[:], in_=ch_t[t])

                q_tile = io.tile([P, d], mybir.dt.float32, tag="q")
                nc.gpsimd.indirect_dma_start(
                    out=q_tile[:],
                    out_offset=None,
                    in_=ht.ap(),
                    in_offset=bass.IndirectOffsetOnAxis(
                        ap=cw_sb[:, t : t + 1], axis=0
                    ),
                )

                d_tile = io.tile([P, d], mybir.dt.float32, tag="d")
                nc.vector.tensor_tensor(
                    out=d_tile[:],
                    in0=ch_tile[:],
                    in1=q_tile[:],
                    op=mybir.AluOpType.subtract,
                )

                sq_tile = scratch.tile([P, d], mybir.dt.float32, tag="sq")
                nc.scalar.activation(
                    out=sq_tile[:],
                    in_=d_tile[:],
                    func=mybir.ActivationFunctionType.Square,
                    accum_out=d2_all[:, t : t + 1],
                )

            dist_all = persist.tile([P, nt], mybir.dt.float32)
            nc.scalar.activation(
                out=dist_all[:],
                in_=d2_all[:],
                func=mybir.ActivationFunctionType.Sqrt,
            )
            nc.scalar.activation(
                out=vals_sb[:],
                in_=dist_all[:],
                func=mybir.ActivationFunctionType.Exp,
                scale=1.0 / SMOOTH,
            )

            nc.sync.dma_start(out=vals.ap(), in_=vals_sb[:])
    nc.compile()
    return nc


def make_in_maps(h_t, ch_sorted, cw_sorted):
    in_maps = []
    for c in range(NCORES):
        sl = slice(c * CSH, (c + 1) * CSH)
        in_maps.append(
            {
                "ht": h_t,
                "ch": ch_sorted[sl],
                "cw": np.ascontiguousarray(cw_sorted[sl].reshape(NT, P).T),
            }
        )
    return in_maps


# ---------------------------------------------------------- shared ------


def finish_on_host(vals_sorted, cw_sorted):
    """segment-sum + log_softmax (tiny O(C)+O(V) work)."""
    p = np.bincount(cw_sorted, weights=vals_sorted.astype(np.float64), minlength=V)
    m = p.max()
    lse = m + np.log(np.exp(p - m).sum())
    return (p - lse).astype(np.float32)[None, :]


def _prep(h_t, cache_h, cache_words):
    h_t = np.ascontiguousarray(np.asarray(h_t), dtype=np.float32)
    cache_h = np.ascontiguousarray(np.asarray(cache_h), dtype=np.float32)
    cw = np.asarray(cache_words).astype(np.int32)
    order = np.argsort(cw, kind="stable")
    return h_t, cache_h[order], cw[order]


def run_device(h_t, ch_sorted, cw_sorted, force_v1=False, verbose=False):
    """Compile + run the SPMD program; returns per-element vals (sorted order)."""
    import time as _time

    _t0 = _time.time()
    v5 = None if force_v1 else prep_v5(cw_sorted)
    if v5 is not None:
        widx_all, rel_all = v5
        nc = build_nc_v5()
        in_maps = make_in_maps_v5(h_t, ch_sorted, widx_all)
    else:
        nc = build_nc()
        in_maps = make_in_maps(h_t, ch_sorted, cw_sorted)
    if verbose:
        print(f"[run_device] build+prep: {_time.time() - _t0:.1f}s")
    _t1 = _time.time()
    res = run_bass_kernel_spmd(nc, in_maps, core_ids=list(range(NCORES)))
    if verbose:
        print(f"[run_device] compile+exec: {_time.time() - _t1:.1f}s")
    if v5 is not None:
        return finish_v5(
            [r["m8"] for r in res.results], rel_all, h_t, ch_sorted, cw_sorted
        )
    return np.concatenate([r["vals"].T.reshape(-1) for r in res.results])


def kernel(h_t, cache_h, cache_words):
    h_t, ch_sorted, cw_sorted = _prep(h_t, cache_h, cache_words)
    vals_sorted = run_device(h_t, ch_sorted, cw_sorted)
    return finish_on_host(vals_sorted, cw_sorted)
